# revision 20
# baseline (speedup 1.0000x reference)
"""Sliding-window causal GQA attention block (QKV proj + RoPE + SDPA + out proj)
on 8 Trainium2 NeuronCores.

Sharding: 8 cores = 2 batches x 4 sequence chunks of 512 tokens. Each core
computes the full attention-block output for its (batch, seq-chunk), including
a 512-token K/V halo recomputed locally (cross-core exchange measured ~150us
fixed collective latency in this runtime -- recompute is far cheaper).

v3: tensor work cut from 1176 to 928 matmuls and DMA dispatch fixed
  - weights/x are pre-tiled on the host so every DMA lands per-partition
    contiguous (4-32KB descriptors instead of 256B; v2's dispatch cost ~1us
    per weight dma_start on the sequencer)
  - RoPE rotate-half via SBUF->SBUF DMA partition swap + sign-folded sin
    (host precomputes sin' = [-sin_lo, +sin_hi]) instead of a perm matmul
  - softmax denominators via gpsimd.partition_all_reduce instead of 96
    ones-vector matmuls + partition_broadcast; AV is copied out of PSUM as
    soon as its accumulation stops so the bank isn't held across the chain
  - attention regrouped as (kv-group, 128-query tile) blocks of N=512
    (4 heads x 128 queries): 5 key tiles per block instead of 6, and only
    the 2 edge tiles (window lower edge, causal diagonal) need mask
    multiplies; out-of-range tiles on chunk-0 cores are killed by a
    per-core exp bias input (-1e5) instead of per-tile masks.

All matmul operands bf16 (fp32 accumulation in PSUM); denominator sums and
reciprocals fp32.
"""
import numpy as np

import concourse.bacc as bacc
import concourse.bass_isa as bass_isa
import concourse.mybir as mybir
import concourse.tile as tile
from concourse.bass_utils import run_bass_kernel_spmd

# Problem constants (hardcoded per contract)
B, S, E = 2, 2048, 2048
H, KV, D = 16, 4, 128
WIN = 512
THETA = 1e6
NCORES = 8
CH = 512          # seq chunk per core
SW = 1024         # K/V window per core (halo 512 + own 512)
P = 128
ECH = E // P      # 16 contraction chunks
NCT = H + 2 * KV  # 24 column tiles of w_qkv
F32 = mybir.dt.float32
BF16 = mybir.dt.bfloat16
SCALE = 1.0 / float(np.sqrt(np.float32(D)))
NEG = -1e5

_CACHE = {}


def _build():
    nc = bacc.Bacc("TRN2", target_bir_lowering=False, debug=False,
                   num_devices=NCORES)

    # pre-tiled inputs: [tile][partition][e-chunk][col] per-partition contiguous
    xtw = nc.dram_tensor("xtw", [P, ECH, SW], BF16, kind="ExternalInput")
    wqkvT = nc.dram_tensor("wqkvT", [NCT, P, ECH, P], BF16, kind="ExternalInput")
    woT = nc.dram_tensor("woT", [ECH, P, ECH, P], BF16, kind="ExternalInput")
    cosw = nc.dram_tensor("cosw", [P, SW], BF16, kind="ExternalInput")
    sinw = nc.dram_tensor("sinw", [P, SW], BF16, kind="ExternalInput")
    maskd = nc.dram_tensor("maskd", [P, CH], BF16, kind="ExternalInput")
    maske = nc.dram_tensor("maske", [P, CH], BF16, kind="ExternalInput")
    biases = nc.dram_tensor("biases", [P, 20], F32, kind="ExternalInput")
    yt = nc.dram_tensor("yt", [E, CH], F32, kind="ExternalOutput")

    with tile.TileContext(nc) as tc:
        with (
            tc.tile_pool(name="res", bufs=1) as res,       # resident tensors
            tc.tile_pool(name="wst", bufs=4) as wst,       # streamed w tiles
            tc.tile_pool(name="rt", bufs=2) as rt,         # rope transients
            tc.tile_pool(name="at", bufs=6) as at,         # attention transients
            tc.tile_pool(name="dt", bufs=4) as dt,         # denom transients
            tc.tile_pool(name="yt_p", bufs=2) as ytp,
            tc.tile_pool(name="pj", bufs=2, space="PSUM") as pj,   # projections
            tc.tile_pool(name="ps", bufs=3, space="PSUM") as ps,   # scores
            tc.tile_pool(name="pa", bufs=3, space="PSUM") as pa,   # AV accum
        ):
            # ---------------- constants (gpsimd queue) ----------------------
            cos_sb = res.tile([P, SW], BF16, tag="cosw")
            sin_sb = res.tile([P, SW], BF16, tag="sinw")
            nc.gpsimd.dma_start(cos_sb[:], cosw.ap())
            nc.gpsimd.dma_start(sin_sb[:], sinw.ap())
            maskd_sb = res.tile([P, CH], BF16, tag="maskd")
            maske_sb = res.tile([P, CH], BF16, tag="maske")
            nc.gpsimd.dma_start(maskd_sb[:], maskd.ap())
            nc.gpsimd.dma_start(maske_sb[:], maske.ap())
            bias_sb = res.tile([P, 20], F32, tag="biases")
            nc.gpsimd.dma_start(bias_sb[:], biases.ap())
            # wv resident [p, kv, e_chunk, 128]
            wv_sb = res.tile([P, KV, ECH, P], BF16, tag="wv")
            for kv in range(KV):
                nc.gpsimd.dma_start(wv_sb[:, kv, :, :], wqkvT.ap()[H + KV + kv])

            # ------------- x window + streamed weights (sync queue) ----------
            def load_w(ct, name):
                t = wst.tile([P, ECH, P], BF16, tag="w", name=name)
                nc.sync.dma_start(t[:], wqkvT.ap()[ct])
                return t

            wk_t = {0: load_w(H, "wk_0")}
            x_sb = res.tile([P, ECH, SW], BF16, tag="x")
            for eh in range(4):
                sl = slice(eh * 4, eh * 4 + 4)
                nc.sync.dma_start(x_sb[:, sl, 0:CH], xtw.ap()[:, sl, 0:CH])
                nc.sync.dma_start(x_sb[:, sl, CH:SW], xtw.ap()[:, sl, CH:SW])

            # ---------------- rope helper ----------------
            # dst = raw*cos + swap64(raw)*sin' ; sin' sign-folded on host.
            def rope(dst, raw_ps, c0, c1, un):
                n = c1 - c0
                raw = rt.tile([P, CH], BF16, tag="raw", name=f"raw_{un}")
                nc.scalar.activation(out=raw[:, :n], in_=raw_ps[:, :n],
                                     func=mybir.ActivationFunctionType.Copy)
                swp = rt.tile([P, CH], BF16, tag="swp", name=f"swp_{un}")
                nc.sync.dma_start(swp[0:64, :n], raw[64:128, :n])
                nc.sync.dma_start(swp[64:128, :n], raw[0:64, :n])
                t1 = rt.tile([P, CH], BF16, tag="t1", name=f"t1_{un}")
                nc.vector.tensor_mul(out=t1[:, :n], in0=raw[:, :n],
                                     in1=cos_sb[:, c0:c1])
                t2 = rt.tile([P, CH], BF16, tag="t2", name=f"t2_{un}")
                nc.vector.tensor_mul(out=t2[:, :n], in0=swp[:, :n],
                                     in1=sin_sb[:, c0:c1])
                nc.vector.tensor_add(out=dst, in0=t1[:, :n], in1=t2[:, :n])

            # During the projection and out-proj phases the attention PSUM
            # pools are idle -- cycle chains across all three pools so the
            # ~1us copy-out latency between chains never stalls the tensor
            # FIFO on a bank.
            pools = [(pj, "pj"), (ps, "sc"), (pa, "pa")]
            pool_i = [0]

            def proj_psum(cols, name):
                pool, tag = pools[pool_i[0] % 3]
                pool_i[0] += 1
                return pool.tile([P, cols], F32, tag=tag, name=name)

            # ------------- K projection (transposed [d, s] layout) -----------
            k_sb = res.tile([P, KV, SW], BF16, tag="k")
            for fk in range(KV):
                if fk > 0:
                    wk_t[fk] = load_w(H + fk, f"wk_{fk}")
                for sh in range(2):
                    k_ps = proj_psum(CH, f"kps_{fk}_{sh}")
                    for e in range(ECH):
                        nc.tensor.matmul(
                            k_ps[:], wk_t[fk][:, e, :],
                            x_sb[:, e, sh * CH:(sh + 1) * CH],
                            start=(e == 0), stop=(e == ECH - 1))
                    rope(k_sb[:, fk, sh * CH:(sh + 1) * CH], k_ps,
                         sh * CH, (sh + 1) * CH, f"k{fk}{sh}")

            # ------------- V projection (natural [s, d] layout) --------------
            # v_sb[:, w, kv*128:(kv+1)*128] = V tile for window token tile w.
            v_sb = res.tile([P, SW // P, KV * D], BF16, tag="v")
            for st in range(SW // P):
                v_ps = proj_psum(KV * D, f"vps_{st}")
                for e in range(ECH):
                    nc.tensor.matmul(v_ps[:], x_sb[:, e, st * P:(st + 1) * P],
                                     wv_sb[:, :, e, :],
                                     start=(e == 0), stop=(e == ECH - 1))
                nc.scalar.activation(out=v_sb[:, st, :], in_=v_ps[:],
                                     func=mybir.ActivationFunctionType.Copy)

            # --- Q proj + attention, software-pipelined units ----------------
            # q_sb block b = kv*4 + qt: [d, cols = head(4) x query(128)]
            # o_sb block b likewise: [vdims(kv), cols = head(4) x query(128)]
            # Unit u emits: Q-proj head u | scores+exp of block u-4 |
            # masks+AV+densum+PAR+avcopy of block u-5 | recip+norm of u-6.
            # The tensor FIFO thus never pops an instruction whose input is
            # still being produced (head-of-line stalls killed), and the exp/
            # vector/PAR chains hide under the Q-proj matmul stream.
            q_sb = res.tile([P, 16, CH], BF16, tag="q")
            o_sb = res.tile([P, 16, CH], BF16, tag="o")
            state = {}
            AVORD = [1, 2, 3, 0, 4]   # unmasked tiles first in the AV chain

            def qproj_head(fi):
                kv, h = fi // 4, fi % 4
                wq_t = load_w(fi, f"wq_{fi}")
                q_ps = pj.tile([P, CH], F32, tag="pj", name=f"qps_{fi}")
                for e in range(ECH):
                    nc.tensor.matmul(q_ps[:], wq_t[:, e, :],
                                     x_sb[:, e, CH:SW],
                                     start=(e == 0), stop=(e == ECH - 1))
                dst = q_sb[:, kv * 4:kv * 4 + 4, h * P:(h + 1) * P]
                rope(dst, q_ps, CH, SW, f"q{fi}")

            def scores_block(blk):
                kv, qt = blk // 4, blk % 4
                pes = []
                for r in range(5):
                    w = qt + r
                    sc_ps = ps.tile([P, CH], F32, tag="sc",
                                    name=f"sc_{blk}_{r}")
                    nc.tensor.matmul(sc_ps[:],
                                     k_sb[:, kv, w * P:(w + 1) * P],
                                     q_sb[:, blk, :],
                                     start=True, stop=True)
                    pe = at.tile([P, CH], BF16, tag="pe",
                                 name=f"pe_{blk}_{r}", bufs=10)
                    nc.scalar.activation(
                        out=pe[:], in_=sc_ps[:],
                        func=mybir.ActivationFunctionType.Exp,
                        scale=SCALE,
                        bias=bias_sb[:, qt * 5 + r:qt * 5 + r + 1])
                    pes.append(pe)
                state[blk] = {"pes": pes}

            def av_block(blk):
                kv, qt = blk // 4, blk % 4
                st = state[blk]
                pts = list(st["pes"])
                for r in (0, 4):
                    pt = at.tile([P, CH], BF16, tag="pt", name=f"pt_{blk}_{r}",
                                 bufs=6)
                    nc.vector.tensor_mul(
                        out=pt[:], in0=pts[r][:],
                        in1=(maske_sb if r == 0 else maskd_sb)[:])
                    pts[r] = pt
                pa_ps = pa.tile([P, CH], F32, tag="pa", name=f"pa_{blk}")
                for i, r in enumerate(AVORD):
                    nc.tensor.matmul(pa_ps[:],
                                     v_sb[:, qt + r, kv * D:(kv + 1) * D],
                                     pts[r][:], start=(i == 0), stop=(i == 4))
                # denominator: sum the 5 pt tiles (vector/gpsimd split as two
                # parallel partial sums), then partition all-reduce
                sA = dt.tile([P, CH], BF16, tag="ds", name=f"dsA_{blk}")
                nc.vector.tensor_add(out=sA[:], in0=pts[1][:], in1=pts[2][:])
                sB = dt.tile([P, CH], BF16, tag="dsb", name=f"dsB_{blk}",
                             bufs=2)
                nc.gpsimd.tensor_add(out=sB[:], in0=pts[0][:], in1=pts[4][:])
                sC = dt.tile([P, CH], BF16, tag="ds", name=f"dsC_{blk}")
                nc.vector.tensor_add(out=sC[:], in0=sA[:], in1=pts[3][:])
                s = dt.tile([P, CH], BF16, tag="dsb", name=f"dsS_{blk}",
                            bufs=2)
                nc.gpsimd.tensor_add(out=s[:], in0=sC[:], in1=sB[:])
                den = dt.tile([P, CH], F32, tag="den", name=f"den_{blk}")
                nc.gpsimd.partition_all_reduce(den[:], s[:], P,
                                               bass_isa.ReduceOp.add)
                st["den"], st["pa"] = den, pa_ps

            def norm_block(blk):
                st = state.pop(blk)
                rc = dt.tile([P, CH], F32, tag="rc", name=f"rc_{blk}")
                nc.vector.reciprocal_approx_fast(out=rc[:], in_=st["den"][:])
                nc.vector.tensor_mul(out=o_sb[:, blk, :],
                                     in0=st["pa"][:], in1=rc[:])

            # Tail units 17..20 have no Q-proj filler: run the first NSPLIT
            # out-proj chains there restricted to f=0..11 (kv0-2 heads, whose
            # o blocks are done), parking partials in SBUF to free the bank.
            NSPLIT = 4
            wo_tiles = {}
            parts = {}

            def of(f):
                kv, h = f // 4, f % 4
                return o_sb[:, kv * 4:kv * 4 + 4, h * P:(h + 1) * P]

            for u in range(22):
                if u < 16:
                    qproj_head(u)
                if 4 <= u < 20:
                    scores_block(u - 4)
                if 5 <= u < 21:
                    av_block(u - 5)
                if 6 <= u < 22:
                    norm_block(u - 6)
                if 17 <= u < 17 + NSPLIT:
                    et = u - 17
                    wo_tiles[et] = wst.tile([P, ECH, P], BF16, tag="wos",
                                            name=f"wos_{et}", bufs=NSPLIT)
                    nc.sync.dma_start(wo_tiles[et][:], woT.ap()[et])
                    yp = pj.tile([P, CH], F32, tag="pj", name=f"yp1_{et}")
                    for f in range(12):
                        nc.tensor.matmul(yp[:], wo_tiles[et][:, f, :], of(f),
                                         start=(f == 0), stop=(f == 11))
                    part = ytp.tile([P, CH], F32, tag="part",
                                    name=f"part_{et}", bufs=NSPLIT)
                    nc.vector.tensor_copy(out=part[:], in_=yp[:])
                    parts[et] = part

            # ------------- out projection, transposed: yt = sum_f woT @ oT ---
            for et in range(ECH):
                y_ps = proj_psum(CH, f"yps_{et}")
                y_sb = ytp.tile([P, CH], F32, tag="ysb", name=f"ysb_{et}")
                if et < NSPLIT:
                    for f in range(12, H):
                        nc.tensor.matmul(y_ps[:], wo_tiles[et][:, f, :], of(f),
                                         start=(f == 12), stop=(f == H - 1))
                    nc.vector.tensor_add(out=y_sb[:], in0=parts[et][:],
                                         in1=y_ps[:])
                else:
                    wo_t = wst.tile([P, ECH, P], BF16, tag="wo",
                                    name=f"wo_{et}", bufs=2)
                    nc.sync.dma_start(wo_t[:], woT.ap()[et])
                    for f in range(H):
                        nc.tensor.matmul(y_ps[:], wo_t[:, f, :], of(f),
                                         start=(f == 0), stop=(f == H - 1))
                    nc.scalar.activation(out=y_sb[:], in_=y_ps[:],
                                         func=mybir.ActivationFunctionType.Copy)
                nc.gpsimd.dma_start(yt.ap()[et * P:(et + 1) * P, :], y_sb[:])

    nc.compile()
    return nc


def _host_constants():
    import ml_dtypes
    inv_freq = (1.0 / (THETA ** (np.arange(0, D, 2, dtype=np.float32) / D))
                ).astype(np.float32)
    ang = np.arange(S, dtype=np.float32)[:, None] * inv_freq[None, :]
    emb = np.concatenate([ang, ang], axis=-1)          # [S, D]
    cos_t = np.ascontiguousarray(np.cos(emb).astype(np.float32).T)  # [D, S]
    sin_t = np.ascontiguousarray(np.sin(emb).astype(np.float32).T)
    sin_t[:D // 2] *= -1.0  # sign-folded for the swap64 rotate-half

    kk = np.arange(P)[:, None]
    qq = np.arange(P)[None, :]
    md = (qq >= kk).astype(np.float32)      # causal diagonal tile
    me = (qq < kk).astype(np.float32)       # window lower-edge tile
    maskd = np.tile(md, (1, 4)).astype(ml_dtypes.bfloat16)
    maske_n = np.tile(me, (1, 4)).astype(ml_dtypes.bfloat16)
    return cos_t, sin_t, maskd, maske_n


def _prepare_in_maps(x, w_qkv, w_o):
    import ml_dtypes
    cos_t, sin_t, maskd, maske_n = _host_constants()
    w_qkv = np.asarray(w_qkv, dtype=np.float32).astype(ml_dtypes.bfloat16)
    w_o = np.asarray(w_o, dtype=np.float32).astype(ml_dtypes.bfloat16)
    # pre-tile: [col-tile, partition, e-chunk, col] (per-partition contiguous)
    wqkvT = np.ascontiguousarray(
        w_qkv.reshape(ECH, P, NCT, P).transpose(2, 1, 0, 3))
    woT = np.ascontiguousarray(
        w_o.reshape(ECH, P, ECH, P).transpose(2, 1, 0, 3))
    maske_z = np.zeros_like(maske_n)
    in_maps = []
    xts = [np.ascontiguousarray(np.asarray(x[b], dtype=np.float32).T
                                ).astype(ml_dtypes.bfloat16)
           for b in range(B)]
    for c in range(NCORES):
        b, chunk = divmod(c, 4)
        s0 = chunk * CH
        xt_win = np.zeros((E, SW), dtype=ml_dtypes.bfloat16)
        cos_win = np.zeros((P, SW), dtype=np.float32)
        sin_win = np.zeros((P, SW), dtype=np.float32)
        lo = s0 - WIN
        src_lo = max(0, lo)
        dst_lo = src_lo - lo
        xt_win[:, dst_lo:] = xts[b][:, src_lo:s0 + CH]
        cos_win[:, dst_lo:] = cos_t[:, src_lo:s0 + CH]
        sin_win[:, dst_lo:] = sin_t[:, src_lo:s0 + CH]
        bias = np.zeros((P, 20), dtype=np.float32)
        if chunk == 0:
            for qt in range(4):
                for r in range(1, 4):
                    if qt + r < 4:          # fully out-of-range key tile
                        bias[:, qt * 5 + r] = NEG
        in_maps.append({
            "xtw": np.ascontiguousarray(
                xt_win.reshape(ECH, P, SW).transpose(1, 0, 2)),
            "wqkvT": wqkvT,
            "woT": woT,
            "cosw": cos_win.astype(ml_dtypes.bfloat16),
            "sinw": sin_win.astype(ml_dtypes.bfloat16),
            "maskd": maskd,
            "maske": maske_z if chunk == 0 else maske_n,
            "biases": bias,
        })
    return in_maps


def _install_ntff_shim():
    """bass_utils wants antenv.axon_hooks for trace=True under axon; this
    environment lacks that module, so synthesize it from the boot helper."""
    import sys
    import types
    if "antenv.axon_hooks" in sys.modules:
        return
    try:
        from trn_agent_boot.trn_boot import _ntff_profile_via_ctypes
        hook = _ntff_profile_via_ctypes("/opt/axon/libaxon_pjrt.so")
    except Exception:
        hook = None
    mod = types.ModuleType("antenv.axon_hooks")
    mod.get_axon_ntff_profile_hook = lambda: hook
    mod.set_axon_ntff_profile_hook = lambda h: None
    sys.modules["antenv.axon_hooks"] = mod


def run(x, w_qkv, w_o, trace=False):
    if "nc" not in _CACHE:
        _CACHE["nc"] = _build()
    nc = _CACHE["nc"]
    in_maps = _prepare_in_maps(np.asarray(x), np.asarray(w_qkv),
                               np.asarray(w_o))
    if trace:
        _install_ntff_shim()
    try:
        res = run_bass_kernel_spmd(nc, in_maps, list(range(NCORES)),
                                   trace=trace)
    except Exception:
        if not trace:
            raise
        res = run_bass_kernel_spmd(nc, in_maps, list(range(NCORES)),
                                   trace=False)
    y = np.empty((B, S, E), dtype=np.float32)
    for c in range(NCORES):
        b, chunk = divmod(c, 4)
        y[b, chunk * CH:(chunk + 1) * CH, :] = res.results[c]["yt"].T
    return y, res


def kernel(x, w_qkv, w_o):
    y, _ = run(x, w_qkv, w_o, trace=False)
    return y


# revision 21
# speedup vs baseline: 1.6156x; 1.6156x over previous
"""Sliding-window causal GQA attention block (QKV proj + RoPE + SDPA + out proj)
on 8 Trainium2 NeuronCores.

Sharding: 8 cores = 2 batches x 4 sequence chunks of 512 tokens. Each core
computes the full attention-block output for its (batch, seq-chunk), including
a 512-token K/V halo recomputed locally (cross-core exchange measured ~150us
fixed collective latency in this runtime -- recompute is far cheaper).

v3: tensor work cut from 1176 to 928 matmuls and DMA dispatch fixed
  - weights/x are pre-tiled on the host so every DMA lands per-partition
    contiguous (4-32KB descriptors instead of 256B; v2's dispatch cost ~1us
    per weight dma_start on the sequencer)
  - RoPE rotate-half via SBUF->SBUF DMA partition swap + sign-folded sin
    (host precomputes sin' = [-sin_lo, +sin_hi]) instead of a perm matmul
  - softmax denominators via gpsimd.partition_all_reduce instead of 96
    ones-vector matmuls + partition_broadcast; AV is copied out of PSUM as
    soon as its accumulation stops so the bank isn't held across the chain
  - attention regrouped as (kv-group, 128-query tile) blocks of N=512
    (4 heads x 128 queries): 5 key tiles per block instead of 6, and only
    the 2 edge tiles (window lower edge, causal diagonal) need mask
    multiplies; out-of-range tiles on chunk-0 cores are killed by a
    per-core exp bias input (-1e5) instead of per-tile masks.

All matmul operands bf16 (fp32 accumulation in PSUM); denominator sums and
reciprocals fp32.
"""
import numpy as np

import concourse.bacc as bacc
import concourse.bass_isa as bass_isa
import concourse.mybir as mybir
import concourse.tile as tile
from concourse.bass_utils import run_bass_kernel_spmd

# Problem constants (hardcoded per contract)
B, S, E = 2, 2048, 2048
H, KV, D = 16, 4, 128
WIN = 512
THETA = 1e6
NCORES = 8
CH = 512          # seq chunk per core
SW = 1024         # K/V window per core (halo 512 + own 512)
P = 128
ECH = E // P      # 16 contraction chunks
NCT = H + 2 * KV  # 24 column tiles of w_qkv
F32 = mybir.dt.float32
BF16 = mybir.dt.bfloat16
SCALE = 1.0 / float(np.sqrt(np.float32(D)))
NEG = -1e5
NEG2 = -1e9

_CACHE = {}


def _build():
    nc = bacc.Bacc("TRN2", target_bir_lowering=False, debug=False,
                   num_devices=NCORES)

    # pre-tiled inputs: [tile][partition][e-chunk][col] per-partition contiguous
    xtw = nc.dram_tensor("xtw", [P, ECH, SW], BF16, kind="ExternalInput")
    wqkvT = nc.dram_tensor("wqkvT", [NCT, P, ECH, P], BF16, kind="ExternalInput")
    woT = nc.dram_tensor("woT", [ECH, P, ECH, P], BF16, kind="ExternalInput")
    cosw = nc.dram_tensor("cosw", [P, SW], BF16, kind="ExternalInput")
    sinw = nc.dram_tensor("sinw", [P, SW], BF16, kind="ExternalInput")
    mid = nc.dram_tensor("mid", [P, P], BF16, kind="ExternalInput")
    mre = nc.dram_tensor("mre", [P, CH], BF16, kind="ExternalInput")
    mrd = nc.dram_tensor("mrd", [P, CH], BF16, kind="ExternalInput")
    biases = nc.dram_tensor("biases", [P, 20], F32, kind="ExternalInput")
    yt = nc.dram_tensor("yt", [E, CH], F32, kind="ExternalOutput")

    with tile.TileContext(nc) as tc:
        with (
            tc.tile_pool(name="res", bufs=1) as res,       # resident tensors
            tc.tile_pool(name="wst", bufs=4) as wst,       # streamed w tiles
            tc.tile_pool(name="rt", bufs=2) as rt,         # rope transients
            tc.tile_pool(name="at", bufs=6) as at,         # attention transients
            tc.tile_pool(name="dt", bufs=4) as dt,         # denom transients
            tc.tile_pool(name="yt_p", bufs=2) as ytp,
            tc.tile_pool(name="pj", bufs=2, space="PSUM") as pj,   # projections
            tc.tile_pool(name="ps", bufs=3, space="PSUM") as ps,   # scores
            tc.tile_pool(name="pa", bufs=3, space="PSUM") as pa,   # AV accum
        ):
            # ---------------- constants (gpsimd queue) ----------------------
            cos_sb = res.tile([P, SW], BF16, tag="cosw")
            sin_sb = res.tile([P, SW], BF16, tag="sinw")
            nc.gpsimd.dma_start(cos_sb[:], cosw.ap())
            nc.gpsimd.dma_start(sin_sb[:], sinw.ap())
            mid_sb = res.tile([P, P], BF16, tag="mid")
            mre_sb = res.tile([P, CH], BF16, tag="mre")
            mrd_sb = res.tile([P, CH], BF16, tag="mrd")
            nc.gpsimd.dma_start(mid_sb[:], mid.ap())
            nc.gpsimd.dma_start(mre_sb[:], mre.ap())
            nc.gpsimd.dma_start(mrd_sb[:], mrd.ap())
            bias_sb = res.tile([P, 20], F32, tag="biases")
            nc.gpsimd.dma_start(bias_sb[:], biases.ap())
            # wv resident [p, kv, e_chunk, 128]
            wv_sb = res.tile([P, KV, ECH, P], BF16, tag="wv")
            for kv in range(KV):
                nc.gpsimd.dma_start(wv_sb[:, kv, :, :], wqkvT.ap()[H + KV + kv])

            # ------------- x window + streamed weights (sync queue) ----------
            def load_w(ct, name):
                t = wst.tile([P, ECH, P], BF16, tag="w", name=name)
                nc.sync.dma_start(t[:], wqkvT.ap()[ct])
                return t

            wk_t = {0: load_w(H, "wk_0")}
            x_sb = res.tile([P, ECH, SW], BF16, tag="x")
            for eh in range(4):
                sl = slice(eh * 4, eh * 4 + 4)
                nc.sync.dma_start(x_sb[:, sl, 0:CH], xtw.ap()[:, sl, 0:CH])
                nc.sync.dma_start(x_sb[:, sl, CH:SW], xtw.ap()[:, sl, CH:SW])

            # ---------------- rope helper ----------------
            # dst = raw*cos + swap64(raw)*sin' ; sin' sign-folded on host.
            def rope(dst, raw_ps, c0, c1, un):
                n = c1 - c0
                raw = rt.tile([P, CH], BF16, tag="raw", name=f"raw_{un}")
                nc.scalar.activation(out=raw[:, :n], in_=raw_ps[:, :n],
                                     func=mybir.ActivationFunctionType.Copy)
                swp = rt.tile([P, CH], BF16, tag="swp", name=f"swp_{un}")
                nc.sync.dma_start(swp[0:64, :n], raw[64:128, :n])
                nc.sync.dma_start(swp[64:128, :n], raw[0:64, :n])
                t1 = rt.tile([P, CH], BF16, tag="t1", name=f"t1_{un}")
                nc.vector.tensor_mul(out=t1[:, :n], in0=raw[:, :n],
                                     in1=cos_sb[:, c0:c1])
                t2 = rt.tile([P, CH], BF16, tag="t2", name=f"t2_{un}")
                nc.vector.tensor_mul(out=t2[:, :n], in0=swp[:, :n],
                                     in1=sin_sb[:, c0:c1])
                nc.vector.tensor_add(out=dst, in0=t1[:, :n], in1=t2[:, :n])

            # During the projection and out-proj phases the attention PSUM
            # pools are idle -- cycle chains across all three pools so the
            # ~1us copy-out latency between chains never stalls the tensor
            # FIFO on a bank.
            pools = [(pj, "pj"), (ps, "sc"), (pa, "pa")]
            pool_i = [0]

            def proj_psum(cols, name):
                pool, tag = pools[pool_i[0] % 3]
                pool_i[0] += 1
                return pool.tile([P, cols], F32, tag=tag, name=name)

            # ------------- K projection (transposed [d, s] layout) -----------
            k_sb = res.tile([P, KV, SW], BF16, tag="k")
            for fk in range(KV):
                if fk > 0:
                    wk_t[fk] = load_w(H + fk, f"wk_{fk}")
                for sh in range(2):
                    k_ps = proj_psum(CH, f"kps_{fk}_{sh}")
                    for e in range(ECH):
                        nc.tensor.matmul(
                            k_ps[:], wk_t[fk][:, e, :],
                            x_sb[:, e, sh * CH:(sh + 1) * CH],
                            start=(e == 0), stop=(e == ECH - 1))
                    rope(k_sb[:, fk, sh * CH:(sh + 1) * CH], k_ps,
                         sh * CH, (sh + 1) * CH, f"k{fk}{sh}")

            # ------------- V projection (natural [s, d] layout) --------------
            # v_sb[:, w, kv*128:(kv+1)*128] = V tile for window token tile w.
            v_sb = res.tile([P, SW // P, KV * D], BF16, tag="v")
            for st in range(SW // P):
                v_ps = proj_psum(KV * D, f"vps_{st}")
                for e in range(ECH):
                    nc.tensor.matmul(v_ps[:], x_sb[:, e, st * P:(st + 1) * P],
                                     wv_sb[:, :, e, :],
                                     start=(e == 0), stop=(e == ECH - 1))
                nc.scalar.activation(out=v_sb[:, st, :], in_=v_ps[:],
                                     func=mybir.ActivationFunctionType.Copy)

            # --- Q proj + attention, software-pipelined units ----------------
            # q_sb block b = kv*4 + qt: [d, cols = head(4) x query(128)]
            # o_sb block b likewise: [vdims(kv), cols = head(4) x query(128)]
            # Unit u emits: Q-proj head u | scores+exp of block u-4 |
            # masks+AV+densum+PAR+avcopy of block u-5 | recip+norm of u-6.
            # The tensor FIFO thus never pops an instruction whose input is
            # still being produced (head-of-line stalls killed), and the exp/
            # vector/PAR chains hide under the Q-proj matmul stream.
            q_sb = res.tile([P, 16, CH], BF16, tag="q")
            o_sb = res.tile([P, 16, CH], BF16, tag="o")
            state = {}
            AVORD = [1, 2, 3, 0, 4]   # unmasked tiles first in the AV chain

            def qproj_head(fi):
                kv, h = fi // 4, fi % 4
                wq_t = load_w(fi, f"wq_{fi}")
                q_ps = pj.tile([P, CH], F32, tag="pj", name=f"qps_{fi}")
                for e in range(ECH):
                    nc.tensor.matmul(q_ps[:], wq_t[:, e, :],
                                     x_sb[:, e, CH:SW],
                                     start=(e == 0), stop=(e == ECH - 1))
                dst = q_sb[:, kv * 4:kv * 4 + 4, h * P:(h + 1) * P]
                rope(dst, q_ps, CH, SW, f"q{fi}")

            def scores_block(blk):
                kv, qt = blk // 4, blk % 4
                pes = []
                for r in range(5):
                    w = qt + r
                    masked = r == 0 or r == 4
                    sc_ps = ps.tile([P, CH], F32, tag="sc",
                                    name=f"sc_{blk}_{r}")
                    nc.tensor.matmul(sc_ps[:],
                                     k_sb[:, kv, w * P:(w + 1) * P],
                                     q_sb[:, blk, :],
                                     start=True, stop=not masked)
                    if masked:
                        nc.tensor.matmul(
                            sc_ps[:], mid_sb[:],
                            (mre_sb if r == 0 else mrd_sb)[:],
                            start=False, stop=True)
                    pe = at.tile([P, CH], BF16, tag="pe",
                                 name=f"pe_{blk}_{r}", bufs=10)
                    nc.scalar.activation(
                        out=pe[:], in_=sc_ps[:],
                        func=mybir.ActivationFunctionType.Exp,
                        scale=SCALE,
                        bias=bias_sb[:, qt * 5 + r:qt * 5 + r + 1])
                    pes.append(pe)
                state[blk] = {"pes": pes}

            def av_block(blk):
                kv, qt = blk // 4, blk % 4
                st = state[blk]
                pes = st["pes"]
                pa_ps = pa.tile([P, CH], F32, tag="pa", name=f"pa_{blk}")
                for r in range(5):
                    nc.tensor.matmul(pa_ps[:],
                                     v_sb[:, qt + r, kv * D:(kv + 1) * D],
                                     pes[r][:], start=(r == 0), stop=(r == 4))
                # denominator: sum the 5 pe tiles, then partition all-reduce
                sA = dt.tile([P, CH], BF16, tag="ds", name=f"dsA_{blk}")
                nc.vector.tensor_add(out=sA[:], in0=pes[0][:], in1=pes[1][:])
                sB = dt.tile([P, CH], BF16, tag="ds", name=f"dsB_{blk}")
                nc.vector.tensor_add(out=sB[:], in0=pes[2][:], in1=pes[3][:])
                sC = dt.tile([P, CH], BF16, tag="ds", name=f"dsC_{blk}")
                nc.vector.tensor_add(out=sC[:], in0=sA[:], in1=sB[:])
                s = dt.tile([P, CH], BF16, tag="ds", name=f"dsS_{blk}")
                nc.vector.tensor_add(out=s[:], in0=sC[:], in1=pes[4][:])
                den = dt.tile([P, CH], F32, tag="den", name=f"den_{blk}")
                nc.gpsimd.partition_all_reduce(den[:], s[:], P,
                                               bass_isa.ReduceOp.add)
                st["den"], st["pa"] = den, pa_ps

            def norm_block(blk):
                st = state.pop(blk)
                rc = dt.tile([P, CH], F32, tag="rc", name=f"rc_{blk}")
                nc.vector.reciprocal_approx_fast(out=rc[:], in_=st["den"][:])
                nc.vector.tensor_mul(out=o_sb[:, blk, :],
                                     in0=st["pa"][:], in1=rc[:])

            # Tail units 17..20 have no Q-proj filler: run the first NSPLIT
            # out-proj chains there restricted to f=0..11 (kv0-2 heads, whose
            # o blocks are done), parking partials in SBUF to free the bank.
            NSPLIT = 4
            wo_tiles = {}
            parts = {}

            def of(f):
                kv, h = f // 4, f % 4
                return o_sb[:, kv * 4:kv * 4 + 4, h * P:(h + 1) * P]

            for u in range(22):
                if u < 16:
                    qproj_head(u)
                if 4 <= u < 20:
                    scores_block(u - 4)
                if 5 <= u < 21:
                    av_block(u - 5)
                if 6 <= u < 22:
                    norm_block(u - 6)
                if 17 <= u < 17 + NSPLIT:
                    et = u - 17
                    wo_tiles[et] = wst.tile([P, ECH, P], BF16, tag="wos",
                                            name=f"wos_{et}", bufs=NSPLIT)
                    nc.sync.dma_start(wo_tiles[et][:], woT.ap()[et])
                    yp = pj.tile([P, CH], F32, tag="pj", name=f"yp1_{et}")
                    for f in range(12):
                        nc.tensor.matmul(yp[:], wo_tiles[et][:, f, :], of(f),
                                         start=(f == 0), stop=(f == 11))
                    part = ytp.tile([P, CH], F32, tag="part",
                                    name=f"part_{et}", bufs=NSPLIT)
                    nc.vector.tensor_copy(out=part[:], in_=yp[:])
                    parts[et] = part

            # ------------- out projection, transposed: yt = sum_f woT @ oT ---
            for et in range(ECH):
                y_ps = proj_psum(CH, f"yps_{et}")
                y_sb = ytp.tile([P, CH], F32, tag="ysb", name=f"ysb_{et}")
                if et < NSPLIT:
                    for f in range(12, H):
                        nc.tensor.matmul(y_ps[:], wo_tiles[et][:, f, :], of(f),
                                         start=(f == 12), stop=(f == H - 1))
                    nc.vector.tensor_add(out=y_sb[:], in0=parts[et][:],
                                         in1=y_ps[:])
                else:
                    wo_t = wst.tile([P, ECH, P], BF16, tag="wo",
                                    name=f"wo_{et}", bufs=2)
                    nc.sync.dma_start(wo_t[:], woT.ap()[et])
                    for f in range(H):
                        nc.tensor.matmul(y_ps[:], wo_t[:, f, :], of(f),
                                         start=(f == 0), stop=(f == H - 1))
                    nc.scalar.activation(out=y_sb[:], in_=y_ps[:],
                                         func=mybir.ActivationFunctionType.Copy)
                nc.gpsimd.dma_start(yt.ap()[et * P:(et + 1) * P, :], y_sb[:])

    nc.compile()
    return nc


def _host_constants():
    import ml_dtypes
    inv_freq = (1.0 / (THETA ** (np.arange(0, D, 2, dtype=np.float32) / D))
                ).astype(np.float32)
    ang = np.arange(S, dtype=np.float32)[:, None] * inv_freq[None, :]
    emb = np.concatenate([ang, ang], axis=-1)          # [S, D]
    cos_t = np.ascontiguousarray(np.cos(emb).astype(np.float32).T)  # [D, S]
    sin_t = np.ascontiguousarray(np.sin(emb).astype(np.float32).T)
    sin_t[:D // 2] *= -1.0  # sign-folded for the swap64 rotate-half

    kk = np.arange(P)[:, None]
    qq = np.arange(P)[None, :]
    # additive score masks (applied via an extra accumulated matmul I^T @ R):
    # R = -1e9 at INVALID (key, query) pairs, 0 at valid.
    rd = np.where(qq < kk, NEG2, 0.0).astype(np.float32)   # causal diagonal
    re_n = np.where(qq >= kk, NEG2, 0.0).astype(np.float32)  # window low edge
    mrd = np.tile(rd, (1, 4)).astype(ml_dtypes.bfloat16)
    mre_n = np.tile(re_n, (1, 4)).astype(ml_dtypes.bfloat16)
    mid = np.eye(P, dtype=np.float32).astype(ml_dtypes.bfloat16)
    return cos_t, sin_t, mid, mrd, mre_n


def _prepare_in_maps(x, w_qkv, w_o):
    import ml_dtypes
    cos_t, sin_t, mid, mrd, mre_n = _host_constants()
    w_qkv = np.asarray(w_qkv, dtype=np.float32).astype(ml_dtypes.bfloat16)
    w_o = np.asarray(w_o, dtype=np.float32).astype(ml_dtypes.bfloat16)
    # pre-tile: [col-tile, partition, e-chunk, col] (per-partition contiguous)
    wqkvT = np.ascontiguousarray(
        w_qkv.reshape(ECH, P, NCT, P).transpose(2, 1, 0, 3))
    woT = np.ascontiguousarray(
        w_o.reshape(ECH, P, ECH, P).transpose(2, 1, 0, 3))
    mre_z = np.full_like(mre_n, NEG2)  # chunk 0: whole edge tile invalid
    in_maps = []
    xts = [np.ascontiguousarray(np.asarray(x[b], dtype=np.float32).T
                                ).astype(ml_dtypes.bfloat16)
           for b in range(B)]
    for c in range(NCORES):
        b, chunk = divmod(c, 4)
        s0 = chunk * CH
        xt_win = np.zeros((E, SW), dtype=ml_dtypes.bfloat16)
        cos_win = np.zeros((P, SW), dtype=np.float32)
        sin_win = np.zeros((P, SW), dtype=np.float32)
        lo = s0 - WIN
        src_lo = max(0, lo)
        dst_lo = src_lo - lo
        xt_win[:, dst_lo:] = xts[b][:, src_lo:s0 + CH]
        cos_win[:, dst_lo:] = cos_t[:, src_lo:s0 + CH]
        sin_win[:, dst_lo:] = sin_t[:, src_lo:s0 + CH]
        bias = np.zeros((P, 20), dtype=np.float32)
        if chunk == 0:
            for qt in range(4):
                for r in range(1, 4):
                    if qt + r < 4:          # fully out-of-range key tile
                        bias[:, qt * 5 + r] = NEG
        in_maps.append({
            "xtw": np.ascontiguousarray(
                xt_win.reshape(ECH, P, SW).transpose(1, 0, 2)),
            "wqkvT": wqkvT,
            "woT": woT,
            "cosw": cos_win.astype(ml_dtypes.bfloat16),
            "sinw": sin_win.astype(ml_dtypes.bfloat16),
            "mid": mid,
            "mrd": mrd,
            "mre": mre_z if chunk == 0 else mre_n,
            "biases": bias,
        })
    return in_maps


def _install_ntff_shim():
    """bass_utils wants antenv.axon_hooks for trace=True under axon; this
    environment lacks that module, so synthesize it from the boot helper."""
    import sys
    import types
    if "antenv.axon_hooks" in sys.modules:
        return
    try:
        from trn_agent_boot.trn_boot import _ntff_profile_via_ctypes
        hook = _ntff_profile_via_ctypes("/opt/axon/libaxon_pjrt.so")
    except Exception:
        hook = None
    mod = types.ModuleType("antenv.axon_hooks")
    mod.get_axon_ntff_profile_hook = lambda: hook
    mod.set_axon_ntff_profile_hook = lambda h: None
    sys.modules["antenv.axon_hooks"] = mod


def run(x, w_qkv, w_o, trace=False):
    if "nc" not in _CACHE:
        _CACHE["nc"] = _build()
    nc = _CACHE["nc"]
    in_maps = _prepare_in_maps(np.asarray(x), np.asarray(w_qkv),
                               np.asarray(w_o))
    if trace:
        _install_ntff_shim()
    try:
        res = run_bass_kernel_spmd(nc, in_maps, list(range(NCORES)),
                                   trace=trace)
    except Exception:
        if not trace:
            raise
        res = run_bass_kernel_spmd(nc, in_maps, list(range(NCORES)),
                                   trace=False)
    y = np.empty((B, S, E), dtype=np.float32)
    for c in range(NCORES):
        b, chunk = divmod(c, 4)
        y[b, chunk * CH:(chunk + 1) * CH, :] = res.results[c]["yt"].T
    return y, res


def kernel(x, w_qkv, w_o):
    y, _ = run(x, w_qkv, w_o, trace=False)
    return y


# revision 27
# speedup vs baseline: 1.6651x; 1.0306x over previous
"""Sliding-window causal GQA attention block (QKV proj + RoPE + SDPA + out proj)
on 8 Trainium2 NeuronCores.

Sharding: 8 cores = 2 batches x 4 sequence chunks of 512 tokens. Each core
computes the full attention-block output for its (batch, seq-chunk), including
a 512-token K/V halo recomputed locally (cross-core exchange measured ~150us
fixed collective latency in this runtime -- recompute is far cheaper).

v3: tensor work cut from 1176 to 928 matmuls and DMA dispatch fixed
  - weights/x are pre-tiled on the host so every DMA lands per-partition
    contiguous (4-32KB descriptors instead of 256B; v2's dispatch cost ~1us
    per weight dma_start on the sequencer)
  - RoPE rotate-half via SBUF->SBUF DMA partition swap + sign-folded sin
    (host precomputes sin' = [-sin_lo, +sin_hi]) instead of a perm matmul
  - softmax denominators via gpsimd.partition_all_reduce instead of 96
    ones-vector matmuls + partition_broadcast; AV is copied out of PSUM as
    soon as its accumulation stops so the bank isn't held across the chain
  - attention regrouped as (kv-group, 128-query tile) blocks of N=512
    (4 heads x 128 queries): 5 key tiles per block instead of 6, and only
    the 2 edge tiles (window lower edge, causal diagonal) need mask
    multiplies; out-of-range tiles on chunk-0 cores are killed by a
    per-core exp bias input (-1e5) instead of per-tile masks.

All matmul operands bf16 (fp32 accumulation in PSUM); denominator sums and
reciprocals fp32.
"""
import numpy as np

import concourse.bacc as bacc
import concourse.bass_isa as bass_isa
import concourse.mybir as mybir
import concourse.tile as tile
from concourse.bass_utils import run_bass_kernel_spmd

# Problem constants (hardcoded per contract)
B, S, E = 2, 2048, 2048
H, KV, D = 16, 4, 128
WIN = 512
THETA = 1e6
NCORES = 8
CH = 512          # seq chunk per core
SW = 1024         # K/V window per core (halo 512 + own 512)
P = 128
ECH = E // P      # 16 contraction chunks
NCT = H + 2 * KV  # 24 column tiles of w_qkv
F32 = mybir.dt.float32
BF16 = mybir.dt.bfloat16
SCALE = 1.0 / float(np.sqrt(np.float32(D)))
NEG = -1e5
NEG2 = -1e9

_CACHE = {}


def _build():
    nc = bacc.Bacc("TRN2", target_bir_lowering=False, debug=False,
                   num_devices=NCORES)

    # pre-tiled inputs: [tile][partition][e-chunk][col] per-partition contiguous
    xtw = nc.dram_tensor("xtw", [P, ECH, SW], BF16, kind="ExternalInput")
    wqkvT = nc.dram_tensor("wqkvT", [NCT, P, ECH, P], BF16, kind="ExternalInput")
    woT = nc.dram_tensor("woT", [ECH, P, ECH, P], BF16, kind="ExternalInput")
    cosw = nc.dram_tensor("cosw", [P, SW], BF16, kind="ExternalInput")
    sinw = nc.dram_tensor("sinw", [P, SW], BF16, kind="ExternalInput")
    mid = nc.dram_tensor("mid", [P, P], BF16, kind="ExternalInput")
    mre = nc.dram_tensor("mre", [P, CH], BF16, kind="ExternalInput")
    mrd = nc.dram_tensor("mrd", [P, CH], BF16, kind="ExternalInput")
    biases = nc.dram_tensor("biases", [P, 20], F32, kind="ExternalInput")
    yt = nc.dram_tensor("yt", [E, CH], F32, kind="ExternalOutput")

    with tile.TileContext(nc) as tc:
        with (
            tc.tile_pool(name="res", bufs=1) as res,       # resident tensors
            tc.tile_pool(name="wst", bufs=4) as wst,       # streamed w tiles
            tc.tile_pool(name="rt", bufs=2) as rt,         # rope transients
            tc.tile_pool(name="at", bufs=6) as at,         # attention transients
            tc.tile_pool(name="dt", bufs=4) as dt,         # denom transients
            tc.tile_pool(name="yt_p", bufs=2) as ytp,
            tc.tile_pool(name="pj", bufs=2, space="PSUM") as pj,   # projections
            tc.tile_pool(name="ps", bufs=3, space="PSUM") as ps,   # scores
            tc.tile_pool(name="pa", bufs=3, space="PSUM") as pa,   # AV accum
        ):
            # ---------------- constants (gpsimd queue) ----------------------
            cos_sb = res.tile([P, SW], BF16, tag="cosw")
            sin_sb = res.tile([P, SW], BF16, tag="sinw")
            nc.gpsimd.dma_start(cos_sb[:], cosw.ap())
            nc.gpsimd.dma_start(sin_sb[:], sinw.ap())
            mid_sb = res.tile([P, P], BF16, tag="mid")
            mre_sb = res.tile([P, CH], BF16, tag="mre")
            mrd_sb = res.tile([P, CH], BF16, tag="mrd")
            nc.gpsimd.dma_start(mid_sb[:], mid.ap())
            nc.gpsimd.dma_start(mre_sb[:], mre.ap())
            nc.gpsimd.dma_start(mrd_sb[:], mrd.ap())
            bias_sb = res.tile([P, 20], F32, tag="biases")
            nc.gpsimd.dma_start(bias_sb[:], biases.ap())
            # wv resident [p, kv, e_chunk, 128]
            wv_sb = res.tile([P, KV, ECH, P], BF16, tag="wv")
            for kv in range(KV):
                nc.gpsimd.dma_start(wv_sb[:, kv, :, :], wqkvT.ap()[H + KV + kv])

            # ------------- x window + streamed weights (sync queue) ----------
            def load_w(ct, name):
                t = wst.tile([P, ECH, P], BF16, tag="w", name=name)
                nc.sync.dma_start(t[:], wqkvT.ap()[ct])
                return t

            wk_t = {0: load_w(H, "wk_0")}
            x_sb = res.tile([P, ECH, SW], BF16, tag="x")
            for eh in range(4):
                sl = slice(eh * 4, eh * 4 + 4)
                nc.sync.dma_start(x_sb[:, sl, 0:CH], xtw.ap()[:, sl, 0:CH])
                nc.sync.dma_start(x_sb[:, sl, CH:SW], xtw.ap()[:, sl, CH:SW])
                if eh < 3:  # interleave K-weight loads into the x stream
                    wk_t[eh + 1] = load_w(H + eh + 1, f"wk_{eh + 1}")

            # ---------------- rope helper ----------------
            # dst = raw*cos + swap64(raw)*sin' ; sin' sign-folded on host.
            def rope(dst, raw_ps, c0, c1, un):
                n = c1 - c0
                raw = rt.tile([P, CH], BF16, tag="raw", name=f"raw_{un}")
                nc.scalar.activation(out=raw[:, :n], in_=raw_ps[:, :n],
                                     func=mybir.ActivationFunctionType.Copy)
                swp = rt.tile([P, CH], BF16, tag="swp", name=f"swp_{un}")
                nc.scalar.dma_start(swp[0:64, :n], raw[64:128, :n])
                nc.scalar.dma_start(swp[64:128, :n], raw[0:64, :n])
                t1 = rt.tile([P, CH], BF16, tag="t1", name=f"t1_{un}")
                nc.vector.tensor_mul(out=t1[:, :n], in0=raw[:, :n],
                                     in1=cos_sb[:, c0:c1])
                t2 = rt.tile([P, CH], BF16, tag="t2", name=f"t2_{un}")
                nc.vector.tensor_mul(out=t2[:, :n], in0=swp[:, :n],
                                     in1=sin_sb[:, c0:c1])
                nc.vector.tensor_add(out=dst, in0=t1[:, :n], in1=t2[:, :n])

            # During the projection and out-proj phases the attention PSUM
            # pools are idle -- cycle chains across all three pools so the
            # ~1us copy-out latency between chains never stalls the tensor
            # FIFO on a bank.
            pools = [(pj, "pj"), (ps, "sc"), (pa, "pa")]
            pool_i = [0]

            def proj_psum(cols, name):
                pool, tag = pools[pool_i[0] % 3]
                pool_i[0] += 1
                return pool.tile([P, cols], F32, tag=tag, name=name)

            # ------------- K projection (transposed [d, s] layout) -----------
            k_sb = res.tile([P, KV, SW], BF16, tag="k")
            for fk in range(KV):
                for sh in range(2):
                    k_ps = proj_psum(CH, f"kps_{fk}_{sh}")
                    for e in range(ECH):
                        nc.tensor.matmul(
                            k_ps[:], wk_t[fk][:, e, :],
                            x_sb[:, e, sh * CH:(sh + 1) * CH],
                            start=(e == 0), stop=(e == ECH - 1))
                    rope(k_sb[:, fk, sh * CH:(sh + 1) * CH], k_ps,
                         sh * CH, (sh + 1) * CH, f"k{fk}{sh}")

            # ------------- V projection (natural [s, d] layout) --------------
            # v_sb[:, w, kv*128:(kv+1)*128] = V tile for window token tile w.
            v_sb = res.tile([P, SW // P, KV * D], BF16, tag="v")
            for st in range(SW // P):
                v_ps = proj_psum(KV * D, f"vps_{st}")
                for e in range(ECH):
                    nc.tensor.matmul(v_ps[:], x_sb[:, e, st * P:(st + 1) * P],
                                     wv_sb[:, :, e, :],
                                     start=(e == 0), stop=(e == ECH - 1))
                nc.scalar.activation(out=v_sb[:, st, :], in_=v_ps[:],
                                     func=mybir.ActivationFunctionType.Copy)

            # --- Q proj + attention, software-pipelined units ----------------
            # q_sb block b = kv*4 + qt: [d, cols = head(4) x query(128)]
            # o_sb block b likewise: [vdims(kv), cols = head(4) x query(128)]
            # Unit u emits: Q-proj head u | scores+exp of block u-4 |
            # masks+AV+densum+PAR+avcopy of block u-5 | recip+norm of u-6.
            # The tensor FIFO thus never pops an instruction whose input is
            # still being produced (head-of-line stalls killed), and the exp/
            # vector/PAR chains hide under the Q-proj matmul stream.
            q_sb = res.tile([P, 16, CH], BF16, tag="q")
            o_sb = res.tile([P, 16, CH], BF16, tag="o")
            state = {}
            AVORD = [1, 2, 3, 0, 4]   # unmasked tiles first in the AV chain

            def qproj_head(fi):
                kv, h = fi // 4, fi % 4
                wq_t = load_w(fi, f"wq_{fi}")
                q_ps = pj.tile([P, CH], F32, tag="pj", name=f"qps_{fi}")
                for e in range(ECH):
                    nc.tensor.matmul(q_ps[:], wq_t[:, e, :],
                                     x_sb[:, e, CH:SW],
                                     start=(e == 0), stop=(e == ECH - 1))
                dst = q_sb[:, kv * 4:kv * 4 + 4, h * P:(h + 1) * P]
                rope(dst, q_ps, CH, SW, f"q{fi}")

            def scores_block(blk):
                kv, qt = blk // 4, blk % 4
                pes = []
                for r in range(5):
                    w = qt + r
                    masked = r == 0 or r == 4
                    sc_ps = ps.tile([P, CH], F32, tag="sc",
                                    name=f"sc_{blk}_{r}")
                    nc.tensor.matmul(sc_ps[:],
                                     k_sb[:, kv, w * P:(w + 1) * P],
                                     q_sb[:, blk, :],
                                     start=True, stop=not masked)
                    if masked:
                        nc.tensor.matmul(
                            sc_ps[:], mid_sb[:],
                            (mre_sb if r == 0 else mrd_sb)[:],
                            start=False, stop=True)
                    pe = at.tile([P, CH], BF16, tag="pe",
                                 name=f"pe_{blk}_{r}", bufs=10)
                    nc.scalar.activation(
                        out=pe[:], in_=sc_ps[:],
                        func=mybir.ActivationFunctionType.Exp,
                        scale=SCALE,
                        bias=bias_sb[:, qt * 5 + r:qt * 5 + r + 1])
                    pes.append(pe)
                state[blk] = {"pes": pes}

            def av_block(blk):
                kv, qt = blk // 4, blk % 4
                st = state[blk]
                pes = st["pes"]
                pa_ps = pa.tile([P, CH], F32, tag="pa", name=f"pa_{blk}")
                for r in range(5):
                    nc.tensor.matmul(pa_ps[:],
                                     v_sb[:, qt + r, kv * D:(kv + 1) * D],
                                     pes[r][:], start=(r == 0), stop=(r == 4))
                # denominator: sum the 5 pe tiles, then partition all-reduce
                sA = dt.tile([P, CH], BF16, tag="ds", name=f"dsA_{blk}")
                nc.vector.tensor_add(out=sA[:], in0=pes[0][:], in1=pes[1][:])
                sB = dt.tile([P, CH], BF16, tag="ds", name=f"dsB_{blk}")
                nc.vector.tensor_add(out=sB[:], in0=pes[2][:], in1=pes[3][:])
                sC = dt.tile([P, CH], BF16, tag="ds", name=f"dsC_{blk}")
                nc.vector.tensor_add(out=sC[:], in0=sA[:], in1=sB[:])
                s = dt.tile([P, CH], BF16, tag="ds", name=f"dsS_{blk}")
                nc.vector.tensor_add(out=s[:], in0=sC[:], in1=pes[4][:])
                den = dt.tile([P, CH], F32, tag="den", name=f"den_{blk}")
                nc.gpsimd.partition_all_reduce(den[:], s[:], P,
                                               bass_isa.ReduceOp.add)
                st["den"], st["pa"] = den, pa_ps

            def norm_block(blk):
                st = state.pop(blk)
                rc = dt.tile([P, CH], F32, tag="rc", name=f"rc_{blk}")
                nc.vector.reciprocal_approx_fast(out=rc[:], in_=st["den"][:])
                nc.vector.tensor_mul(out=o_sb[:, blk, :],
                                     in0=st["pa"][:], in1=rc[:])

            # Tail units 16..21 have no Q-proj filler: run the first NSPLIT
            # out-proj chains there restricted to the f tiles whose o blocks
            # are already done, parking partials in SBUF to free the bank.
            NSPLIT = 6
            FSPLIT = {0: 8, 1: 12, 2: 12, 3: 12, 4: 12, 5: 12}
            wo_tiles = {}
            parts = {}

            def of(f):
                kv, h = f // 4, f % 4
                return o_sb[:, kv * 4:kv * 4 + 4, h * P:(h + 1) * P]

            for u in range(22):
                if u < 16:
                    qproj_head(u)
                if 4 <= u < 20:
                    scores_block(u - 4)
                if 5 <= u < 21:
                    av_block(u - 5)
                if 6 <= u < 22:
                    norm_block(u - 6)
                if 16 <= u < 16 + NSPLIT:
                    et = u - 16
                    fs = FSPLIT[et]
                    wo_tiles[et] = wst.tile([P, ECH, P], BF16, tag="wos",
                                            name=f"wos_{et}", bufs=NSPLIT)
                    nc.sync.dma_start(wo_tiles[et][:], woT.ap()[et])
                    yp = pj.tile([P, CH], F32, tag="pj", name=f"yp1_{et}")
                    for f in range(fs):
                        nc.tensor.matmul(yp[:], wo_tiles[et][:, f, :], of(f),
                                         start=(f == 0), stop=(f == fs - 1))
                    part = ytp.tile([P, CH], F32, tag="part",
                                    name=f"part_{et}", bufs=NSPLIT)
                    nc.vector.tensor_copy(out=part[:], in_=yp[:])
                    parts[et] = part

            # ------------- out projection, transposed: yt = sum_f woT @ oT ---
            for et in range(ECH):
                y_ps = proj_psum(CH, f"yps_{et}")
                y_sb = ytp.tile([P, CH], F32, tag="ysb", name=f"ysb_{et}")
                if et < NSPLIT:
                    fs = FSPLIT[et]
                    for f in range(fs, H):
                        nc.tensor.matmul(y_ps[:], wo_tiles[et][:, f, :], of(f),
                                         start=(f == fs), stop=(f == H - 1))
                    nc.vector.tensor_add(out=y_sb[:], in0=parts[et][:],
                                         in1=y_ps[:])
                else:
                    wo_t = wst.tile([P, ECH, P], BF16, tag="wo",
                                    name=f"wo_{et}", bufs=2)
                    nc.sync.dma_start(wo_t[:], woT.ap()[et])
                    for f in range(H):
                        nc.tensor.matmul(y_ps[:], wo_t[:, f, :], of(f),
                                         start=(f == 0), stop=(f == H - 1))
                    nc.scalar.activation(out=y_sb[:], in_=y_ps[:],
                                         func=mybir.ActivationFunctionType.Copy)
                nc.gpsimd.dma_start(yt.ap()[et * P:(et + 1) * P, :], y_sb[:])

    nc.compile()
    return nc


def _host_constants():
    import ml_dtypes
    inv_freq = (1.0 / (THETA ** (np.arange(0, D, 2, dtype=np.float32) / D))
                ).astype(np.float32)
    ang = np.arange(S, dtype=np.float32)[:, None] * inv_freq[None, :]
    emb = np.concatenate([ang, ang], axis=-1)          # [S, D]
    cos_t = np.ascontiguousarray(np.cos(emb).astype(np.float32).T)  # [D, S]
    sin_t = np.ascontiguousarray(np.sin(emb).astype(np.float32).T)
    sin_t[:D // 2] *= -1.0  # sign-folded for the swap64 rotate-half

    kk = np.arange(P)[:, None]
    qq = np.arange(P)[None, :]
    # additive score masks (applied via an extra accumulated matmul I^T @ R):
    # R = -1e9 at INVALID (key, query) pairs, 0 at valid.
    rd = np.where(qq < kk, NEG2, 0.0).astype(np.float32)   # causal diagonal
    re_n = np.where(qq >= kk, NEG2, 0.0).astype(np.float32)  # window low edge
    mrd = np.tile(rd, (1, 4)).astype(ml_dtypes.bfloat16)
    mre_n = np.tile(re_n, (1, 4)).astype(ml_dtypes.bfloat16)
    mid = np.eye(P, dtype=np.float32).astype(ml_dtypes.bfloat16)
    return cos_t, sin_t, mid, mrd, mre_n


def _prepare_in_maps(x, w_qkv, w_o):
    import ml_dtypes
    cos_t, sin_t, mid, mrd, mre_n = _host_constants()
    w_qkv = np.asarray(w_qkv, dtype=np.float32).astype(ml_dtypes.bfloat16)
    w_o = np.asarray(w_o, dtype=np.float32).astype(ml_dtypes.bfloat16)
    # pre-tile: [col-tile, partition, e-chunk, col] (per-partition contiguous)
    wqkvT = np.ascontiguousarray(
        w_qkv.reshape(ECH, P, NCT, P).transpose(2, 1, 0, 3))
    woT = np.ascontiguousarray(
        w_o.reshape(ECH, P, ECH, P).transpose(2, 1, 0, 3))
    mre_z = np.full_like(mre_n, NEG2)  # chunk 0: whole edge tile invalid
    in_maps = []
    xts = [np.ascontiguousarray(np.asarray(x[b], dtype=np.float32).T
                                ).astype(ml_dtypes.bfloat16)
           for b in range(B)]
    for c in range(NCORES):
        b, chunk = divmod(c, 4)
        s0 = chunk * CH
        xt_win = np.zeros((E, SW), dtype=ml_dtypes.bfloat16)
        cos_win = np.zeros((P, SW), dtype=np.float32)
        sin_win = np.zeros((P, SW), dtype=np.float32)
        lo = s0 - WIN
        src_lo = max(0, lo)
        dst_lo = src_lo - lo
        xt_win[:, dst_lo:] = xts[b][:, src_lo:s0 + CH]
        cos_win[:, dst_lo:] = cos_t[:, src_lo:s0 + CH]
        sin_win[:, dst_lo:] = sin_t[:, src_lo:s0 + CH]
        bias = np.zeros((P, 20), dtype=np.float32)
        if chunk == 0:
            for qt in range(4):
                for r in range(1, 4):
                    if qt + r < 4:          # fully out-of-range key tile
                        bias[:, qt * 5 + r] = NEG
        in_maps.append({
            "xtw": np.ascontiguousarray(
                xt_win.reshape(ECH, P, SW).transpose(1, 0, 2)),
            "wqkvT": wqkvT,
            "woT": woT,
            "cosw": cos_win.astype(ml_dtypes.bfloat16),
            "sinw": sin_win.astype(ml_dtypes.bfloat16),
            "mid": mid,
            "mrd": mrd,
            "mre": mre_z if chunk == 0 else mre_n,
            "biases": bias,
        })
    return in_maps


def _install_ntff_shim():
    """bass_utils wants antenv.axon_hooks for trace=True under axon; this
    environment lacks that module, so synthesize it from the boot helper."""
    import sys
    import types
    if "antenv.axon_hooks" in sys.modules:
        return
    try:
        from trn_agent_boot.trn_boot import _ntff_profile_via_ctypes
        hook = _ntff_profile_via_ctypes("/opt/axon/libaxon_pjrt.so")
    except Exception:
        hook = None
    mod = types.ModuleType("antenv.axon_hooks")
    mod.get_axon_ntff_profile_hook = lambda: hook
    mod.set_axon_ntff_profile_hook = lambda h: None
    sys.modules["antenv.axon_hooks"] = mod


def run(x, w_qkv, w_o, trace=False):
    if "nc" not in _CACHE:
        _CACHE["nc"] = _build()
    nc = _CACHE["nc"]
    in_maps = _prepare_in_maps(np.asarray(x), np.asarray(w_qkv),
                               np.asarray(w_o))
    if trace:
        _install_ntff_shim()
    try:
        res = run_bass_kernel_spmd(nc, in_maps, list(range(NCORES)),
                                   trace=trace)
    except Exception:
        if not trace:
            raise
        res = run_bass_kernel_spmd(nc, in_maps, list(range(NCORES)),
                                   trace=False)
    y = np.empty((B, S, E), dtype=np.float32)
    for c in range(NCORES):
        b, chunk = divmod(c, 4)
        y[b, chunk * CH:(chunk + 1) * CH, :] = res.results[c]["yt"].T
    return y, res


def kernel(x, w_qkv, w_o):
    y, _ = run(x, w_qkv, w_o, trace=False)
    return y


# revision 31
# speedup vs baseline: 1.7386x; 1.0442x over previous
"""Sliding-window causal GQA attention block (QKV proj + RoPE + SDPA + out proj)
on 8 Trainium2 NeuronCores.

Sharding: 8 cores = 2 batches x 4 sequence chunks of 512 tokens. Each core
computes the full attention-block output for its (batch, seq-chunk), including
a 512-token K/V halo recomputed locally (cross-core exchange measured ~150us
fixed collective latency in this runtime -- recompute is far cheaper).

v3: tensor work cut from 1176 to 928 matmuls and DMA dispatch fixed
  - weights/x are pre-tiled on the host so every DMA lands per-partition
    contiguous (4-32KB descriptors instead of 256B; v2's dispatch cost ~1us
    per weight dma_start on the sequencer)
  - RoPE rotate-half via SBUF->SBUF DMA partition swap + sign-folded sin
    (host precomputes sin' = [-sin_lo, +sin_hi]) instead of a perm matmul
  - softmax denominators via gpsimd.partition_all_reduce instead of 96
    ones-vector matmuls + partition_broadcast; AV is copied out of PSUM as
    soon as its accumulation stops so the bank isn't held across the chain
  - attention regrouped as (kv-group, 128-query tile) blocks of N=512
    (4 heads x 128 queries): 5 key tiles per block instead of 6, and only
    the 2 edge tiles (window lower edge, causal diagonal) need mask
    multiplies; out-of-range tiles on chunk-0 cores are killed by a
    per-core exp bias input (-1e5) instead of per-tile masks.

All matmul operands bf16 (fp32 accumulation in PSUM); denominator sums and
reciprocals fp32.
"""
import numpy as np

import concourse.bacc as bacc
import concourse.bass_isa as bass_isa
import concourse.mybir as mybir
import concourse.tile as tile
from concourse.bass_utils import run_bass_kernel_spmd

# Problem constants (hardcoded per contract)
B, S, E = 2, 2048, 2048
H, KV, D = 16, 4, 128
WIN = 512
THETA = 1e6
NCORES = 8
CH = 512          # seq chunk per core
SW = 1024         # K/V window per core (halo 512 + own 512)
P = 128
ECH = E // P      # 16 contraction chunks
NCT = H + 2 * KV  # 24 column tiles of w_qkv
F32 = mybir.dt.float32
BF16 = mybir.dt.bfloat16
SCALE = 1.0 / float(np.sqrt(np.float32(D)))
NEG = -1e5
NEG2 = -1e9

_CACHE = {}


def _build():
    nc = bacc.Bacc("TRN2", target_bir_lowering=False, debug=False,
                   num_devices=NCORES)

    # pre-tiled inputs: [tile][partition][e-chunk][col] per-partition contiguous
    xtw = nc.dram_tensor("xtw", [P, ECH, SW], BF16, kind="ExternalInput")
    wqkvT = nc.dram_tensor("wqkvT", [NCT, P, ECH, P], BF16, kind="ExternalInput")
    woT = nc.dram_tensor("woT", [ECH, P, ECH, P], BF16, kind="ExternalInput")
    cosw = nc.dram_tensor("cosw", [P, SW], BF16, kind="ExternalInput")
    sinw = nc.dram_tensor("sinw", [P, SW], BF16, kind="ExternalInput")
    mid = nc.dram_tensor("mid", [P, P], BF16, kind="ExternalInput")
    mre = nc.dram_tensor("mre", [P, CH], BF16, kind="ExternalInput")
    mrd = nc.dram_tensor("mrd", [P, CH], BF16, kind="ExternalInput")
    biases = nc.dram_tensor("biases", [P, 20], F32, kind="ExternalInput")
    yt = nc.dram_tensor("yt", [E, CH], F32, kind="ExternalOutput")

    with tile.TileContext(nc) as tc:
        with (
            tc.tile_pool(name="res", bufs=1) as res,       # resident tensors
            tc.tile_pool(name="wst", bufs=4) as wst,       # streamed w tiles
            tc.tile_pool(name="rt", bufs=2) as rt,         # rope transients
            tc.tile_pool(name="at", bufs=6) as at,         # attention transients
            tc.tile_pool(name="dt", bufs=4) as dt,         # denom transients
            tc.tile_pool(name="yt_p", bufs=2) as ytp,
            tc.tile_pool(name="pj", bufs=2, space="PSUM") as pj,   # projections
            tc.tile_pool(name="ps", bufs=3, space="PSUM") as ps,   # scores
            tc.tile_pool(name="pa", bufs=3, space="PSUM") as pa,   # AV accum
        ):
            # ---------------- constants (gpsimd queue) ----------------------
            cos_sb = res.tile([P, SW], BF16, tag="cosw")
            sin_sb = res.tile([P, SW], BF16, tag="sinw")
            nc.gpsimd.dma_start(cos_sb[:], cosw.ap())
            nc.gpsimd.dma_start(sin_sb[:], sinw.ap())
            mid_sb = res.tile([P, P], BF16, tag="mid")
            mre_sb = res.tile([P, CH], BF16, tag="mre")
            mrd_sb = res.tile([P, CH], BF16, tag="mrd")
            nc.gpsimd.dma_start(mid_sb[:], mid.ap())
            nc.gpsimd.dma_start(mre_sb[:], mre.ap())
            nc.gpsimd.dma_start(mrd_sb[:], mrd.ap())
            bias_sb = res.tile([P, 20], F32, tag="biases")
            nc.gpsimd.dma_start(bias_sb[:], biases.ap())
            # wv resident [p, kv, e_chunk, 128]
            wv_sb = res.tile([P, KV, ECH, P], BF16, tag="wv")
            for kv in range(KV):
                nc.gpsimd.dma_start(wv_sb[:, kv, :, :], wqkvT.ap()[H + KV + kv])

            # ------------- x window + streamed weights (sync queue) ----------
            def load_w(ct, name):
                t = wst.tile([P, ECH, P], BF16, tag="w", name=name)
                nc.sync.dma_start(t[:], wqkvT.ap()[ct])
                return t

            wk_t = {0: load_w(H, "wk_0")}
            x_sb = res.tile([P, ECH, SW], BF16, tag="x")
            for eh in range(4):
                sl = slice(eh * 4, eh * 4 + 4)
                nc.sync.dma_start(x_sb[:, sl, 0:CH], xtw.ap()[:, sl, 0:CH])
                nc.sync.dma_start(x_sb[:, sl, CH:SW], xtw.ap()[:, sl, CH:SW])
                if eh < 3:  # interleave K-weight loads into the x stream
                    wk_t[eh + 1] = load_w(H + eh + 1, f"wk_{eh + 1}")

            # ---------------- rope helpers ----------------
            # dst = raw*cos + swap64(raw)*sin' ; sin' sign-folded on host.
            # Split in two so the vector muls can be emitted late in a unit
            # (after the attention vector work whose inputs are ready first).
            def rope_start(raw_ps, c0, c1, un):
                n = c1 - c0
                raw = rt.tile([P, CH], BF16, tag="raw", name=f"raw_{un}")
                nc.scalar.activation(out=raw[:, :n], in_=raw_ps[:, :n],
                                     func=mybir.ActivationFunctionType.Copy)
                swp = rt.tile([P, CH], BF16, tag="swp", name=f"swp_{un}")
                nc.scalar.dma_start(swp[0:64, :n], raw[64:128, :n])
                nc.scalar.dma_start(swp[64:128, :n], raw[0:64, :n])
                return raw, swp, c0, c1, un

            def rope_finish(dst, rs):
                raw, swp, c0, c1, un = rs
                n = c1 - c0
                t1 = rt.tile([P, CH], BF16, tag="t1", name=f"t1_{un}")
                nc.vector.tensor_mul(out=t1[:, :n], in0=raw[:, :n],
                                     in1=cos_sb[:, c0:c1])
                t2 = rt.tile([P, CH], BF16, tag="t2", name=f"t2_{un}")
                nc.vector.tensor_mul(out=t2[:, :n], in0=swp[:, :n],
                                     in1=sin_sb[:, c0:c1])
                nc.vector.tensor_add(out=dst, in0=t1[:, :n], in1=t2[:, :n])

            def rope(dst, raw_ps, c0, c1, un):
                rope_finish(dst, rope_start(raw_ps, c0, c1, un))

            # During the projection and out-proj phases the attention PSUM
            # pools are idle -- cycle chains across all three pools so the
            # ~1us copy-out latency between chains never stalls the tensor
            # FIFO on a bank.
            pools = [(pj, "pj"), (ps, "sc"), (pa, "pa")]
            pool_i = [0]

            def proj_psum(cols, name):
                pool, tag = pools[pool_i[0] % 3]
                pool_i[0] += 1
                return pool.tile([P, cols], F32, tag=tag, name=name)

            # ------------- K projection (transposed [d, s] layout) -----------
            k_sb = res.tile([P, KV, SW], BF16, tag="k")
            for fk in range(KV):
                for sh in range(2):
                    k_ps = proj_psum(CH, f"kps_{fk}_{sh}")
                    for e in range(ECH):
                        nc.tensor.matmul(
                            k_ps[:], wk_t[fk][:, e, :],
                            x_sb[:, e, sh * CH:(sh + 1) * CH],
                            start=(e == 0), stop=(e == ECH - 1))
                    rope(k_sb[:, fk, sh * CH:(sh + 1) * CH], k_ps,
                         sh * CH, (sh + 1) * CH, f"k{fk}{sh}")

            # ------------- V projection (natural [s, d] layout) --------------
            # v_sb[:, w, kv*128:(kv+1)*128] = V tile for window token tile w.
            v_sb = res.tile([P, SW // P, KV * D], BF16, tag="v")
            for st in range(SW // P):
                v_ps = proj_psum(KV * D, f"vps_{st}")
                for e in range(ECH):
                    nc.tensor.matmul(v_ps[:], x_sb[:, e, st * P:(st + 1) * P],
                                     wv_sb[:, :, e, :],
                                     start=(e == 0), stop=(e == ECH - 1))
                nc.scalar.activation(out=v_sb[:, st, :], in_=v_ps[:],
                                     func=mybir.ActivationFunctionType.Copy)

            # --- Q proj + attention, software-pipelined units ----------------
            # q_sb block b = kv*4 + qt: [d, cols = head(4) x query(128)]
            # o_sb block b likewise: [vdims(kv), cols = head(4) x query(128)]
            # Unit u emits: Q-proj head u | scores+exp of block u-4 |
            # masks+AV+densum+PAR+avcopy of block u-5 | recip+norm of u-6.
            # The tensor FIFO thus never pops an instruction whose input is
            # still being produced (head-of-line stalls killed), and the exp/
            # vector/PAR chains hide under the Q-proj matmul stream.
            q_sb = res.tile([P, 16, CH], BF16, tag="q")
            o_sb = res.tile([P, 16, CH], BF16, tag="o")
            state = {}
            AVORD = [1, 2, 3, 0, 4]   # unmasked tiles first in the AV chain

            def qproj_head(fi):
                kv, h = fi // 4, fi % 4
                wq_t = load_w(fi, f"wq_{fi}")
                q_ps = pj.tile([P, CH], F32, tag="pj", name=f"qps_{fi}")
                for e in range(ECH):
                    nc.tensor.matmul(q_ps[:], wq_t[:, e, :],
                                     x_sb[:, e, CH:SW],
                                     start=(e == 0), stop=(e == ECH - 1))
                dst = q_sb[:, kv * 4:kv * 4 + 4, h * P:(h + 1) * P]
                return dst, rope_start(q_ps, CH, SW, f"q{fi}")

            def scores_block(blk):
                kv, qt = blk // 4, blk % 4
                pes = []
                for r in range(5):
                    w = qt + r
                    masked = r == 0 or r == 4
                    sc_ps = ps.tile([P, CH], F32, tag="sc",
                                    name=f"sc_{blk}_{r}")
                    nc.tensor.matmul(sc_ps[:],
                                     k_sb[:, kv, w * P:(w + 1) * P],
                                     q_sb[:, blk, :],
                                     start=True, stop=not masked)
                    if masked:
                        nc.tensor.matmul(
                            sc_ps[:], mid_sb[:],
                            (mre_sb if r == 0 else mrd_sb)[:],
                            start=False, stop=True)
                    pe = at.tile([P, CH], BF16, tag="pe",
                                 name=f"pe_{blk}_{r}", bufs=10)
                    nc.scalar.activation(
                        out=pe[:], in_=sc_ps[:],
                        func=mybir.ActivationFunctionType.Exp,
                        scale=SCALE,
                        bias=bias_sb[:, qt * 5 + r:qt * 5 + r + 1])
                    pes.append(pe)
                state[blk] = {"pes": pes}

            def av_block(blk):
                kv, qt = blk // 4, blk % 4
                st = state[blk]
                pes = st["pes"]
                pa_ps = pa.tile([P, CH], F32, tag="pa", name=f"pa_{blk}")
                for r in range(5):
                    nc.tensor.matmul(pa_ps[:],
                                     v_sb[:, qt + r, kv * D:(kv + 1) * D],
                                     pes[r][:], start=(r == 0), stop=(r == 4))
                # denominator: sum the 5 pe tiles, then partition all-reduce
                sA = dt.tile([P, CH], BF16, tag="ds", name=f"dsA_{blk}")
                nc.vector.tensor_add(out=sA[:], in0=pes[0][:], in1=pes[1][:])
                sB = dt.tile([P, CH], BF16, tag="ds", name=f"dsB_{blk}")
                nc.vector.tensor_add(out=sB[:], in0=pes[2][:], in1=pes[3][:])
                sC = dt.tile([P, CH], BF16, tag="ds", name=f"dsC_{blk}")
                nc.vector.tensor_add(out=sC[:], in0=sA[:], in1=sB[:])
                s = dt.tile([P, CH], BF16, tag="ds", name=f"dsS_{blk}")
                nc.vector.tensor_add(out=s[:], in0=sC[:], in1=pes[4][:])
                den = dt.tile([P, CH], F32, tag="den", name=f"den_{blk}")
                nc.gpsimd.partition_all_reduce(den[:], s[:], P,
                                               bass_isa.ReduceOp.add)
                st["den"], st["pa"] = den, pa_ps

            def norm_block(blk):
                st = state.pop(blk)
                rc = dt.tile([P, CH], F32, tag="rc", name=f"rc_{blk}")
                nc.vector.reciprocal_approx_fast(out=rc[:], in_=st["den"][:])
                nc.vector.tensor_mul(out=o_sb[:, blk, :],
                                     in0=st["pa"][:], in1=rc[:])

            # Tail units 16..21 have no Q-proj filler: run the first NSPLIT
            # out-proj chains there restricted to the f tiles whose o blocks
            # are already done, parking partials in SBUF to free the bank.
            NSPLIT = 6
            FSPLIT = {0: 8, 1: 12, 2: 12, 3: 12, 4: 12, 5: 12}
            wo_tiles = {}
            parts = {}

            def of(f):
                kv, h = f // 4, f % 4
                return o_sb[:, kv * 4:kv * 4 + 4, h * P:(h + 1) * P]

            for u in range(22):
                pend_rope = qproj_head(u) if u < 16 else None
                if 4 <= u < 20:
                    scores_block(u - 4)
                if 5 <= u < 21:
                    av_block(u - 5)
                if 6 <= u < 22:
                    norm_block(u - 6)
                if pend_rope is not None:
                    rope_finish(*pend_rope)
                if 16 <= u < 16 + NSPLIT:
                    et = u - 16
                    fs = FSPLIT[et]
                    wo_tiles[et] = wst.tile([P, ECH, P], BF16, tag="wos",
                                            name=f"wos_{et}", bufs=NSPLIT)
                    nc.sync.dma_start(wo_tiles[et][:], woT.ap()[et])
                    yp = pj.tile([P, CH], F32, tag="pj", name=f"yp1_{et}")
                    for f in range(fs):
                        nc.tensor.matmul(yp[:], wo_tiles[et][:, f, :], of(f),
                                         start=(f == 0), stop=(f == fs - 1))
                    part = ytp.tile([P, CH], F32, tag="part",
                                    name=f"part_{et}", bufs=NSPLIT)
                    nc.scalar.activation(out=part[:], in_=yp[:],
                                         func=mybir.ActivationFunctionType.Copy)
                    parts[et] = part

            # ------------- out projection, transposed: yt = sum_f woT @ oT ---
            for et in range(ECH):
                y_ps = proj_psum(CH, f"yps_{et}")
                y_sb = ytp.tile([P, CH], F32, tag="ysb", name=f"ysb_{et}")
                if et < NSPLIT:
                    fs = FSPLIT[et]
                    for f in range(fs, H):
                        nc.tensor.matmul(y_ps[:], wo_tiles[et][:, f, :], of(f),
                                         start=(f == fs), stop=(f == H - 1))
                    nc.vector.tensor_add(out=y_sb[:], in0=parts[et][:],
                                         in1=y_ps[:])
                else:
                    wo_t = wst.tile([P, ECH, P], BF16, tag="wo",
                                    name=f"wo_{et}", bufs=2)
                    nc.sync.dma_start(wo_t[:], woT.ap()[et])
                    for f in range(H):
                        nc.tensor.matmul(y_ps[:], wo_t[:, f, :], of(f),
                                         start=(f == 0), stop=(f == H - 1))
                    nc.scalar.activation(out=y_sb[:], in_=y_ps[:],
                                         func=mybir.ActivationFunctionType.Copy)
                nc.gpsimd.dma_start(yt.ap()[et * P:(et + 1) * P, :], y_sb[:])

    nc.compile()
    return nc


def _host_constants():
    import ml_dtypes
    inv_freq = (1.0 / (THETA ** (np.arange(0, D, 2, dtype=np.float32) / D))
                ).astype(np.float32)
    ang = np.arange(S, dtype=np.float32)[:, None] * inv_freq[None, :]
    emb = np.concatenate([ang, ang], axis=-1)          # [S, D]
    cos_t = np.ascontiguousarray(np.cos(emb).astype(np.float32).T)  # [D, S]
    sin_t = np.ascontiguousarray(np.sin(emb).astype(np.float32).T)
    sin_t[:D // 2] *= -1.0  # sign-folded for the swap64 rotate-half

    kk = np.arange(P)[:, None]
    qq = np.arange(P)[None, :]
    # additive score masks (applied via an extra accumulated matmul I^T @ R):
    # R = -1e9 at INVALID (key, query) pairs, 0 at valid.
    rd = np.where(qq < kk, NEG2, 0.0).astype(np.float32)   # causal diagonal
    re_n = np.where(qq >= kk, NEG2, 0.0).astype(np.float32)  # window low edge
    mrd = np.tile(rd, (1, 4)).astype(ml_dtypes.bfloat16)
    mre_n = np.tile(re_n, (1, 4)).astype(ml_dtypes.bfloat16)
    mid = np.eye(P, dtype=np.float32).astype(ml_dtypes.bfloat16)
    return cos_t, sin_t, mid, mrd, mre_n


def _prepare_in_maps(x, w_qkv, w_o):
    import ml_dtypes
    cos_t, sin_t, mid, mrd, mre_n = _host_constants()
    w_qkv = np.asarray(w_qkv, dtype=np.float32).astype(ml_dtypes.bfloat16)
    w_o = np.asarray(w_o, dtype=np.float32).astype(ml_dtypes.bfloat16)
    # pre-tile: [col-tile, partition, e-chunk, col] (per-partition contiguous)
    wqkvT = np.ascontiguousarray(
        w_qkv.reshape(ECH, P, NCT, P).transpose(2, 1, 0, 3))
    woT = np.ascontiguousarray(
        w_o.reshape(ECH, P, ECH, P).transpose(2, 1, 0, 3))
    mre_z = np.full_like(mre_n, NEG2)  # chunk 0: whole edge tile invalid
    in_maps = []
    xts = [np.ascontiguousarray(np.asarray(x[b], dtype=np.float32).T
                                ).astype(ml_dtypes.bfloat16)
           for b in range(B)]
    for c in range(NCORES):
        b, chunk = divmod(c, 4)
        s0 = chunk * CH
        xt_win = np.zeros((E, SW), dtype=ml_dtypes.bfloat16)
        cos_win = np.zeros((P, SW), dtype=np.float32)
        sin_win = np.zeros((P, SW), dtype=np.float32)
        lo = s0 - WIN
        src_lo = max(0, lo)
        dst_lo = src_lo - lo
        xt_win[:, dst_lo:] = xts[b][:, src_lo:s0 + CH]
        cos_win[:, dst_lo:] = cos_t[:, src_lo:s0 + CH]
        sin_win[:, dst_lo:] = sin_t[:, src_lo:s0 + CH]
        bias = np.zeros((P, 20), dtype=np.float32)
        if chunk == 0:
            for qt in range(4):
                for r in range(1, 4):
                    if qt + r < 4:          # fully out-of-range key tile
                        bias[:, qt * 5 + r] = NEG
        in_maps.append({
            "xtw": np.ascontiguousarray(
                xt_win.reshape(ECH, P, SW).transpose(1, 0, 2)),
            "wqkvT": wqkvT,
            "woT": woT,
            "cosw": cos_win.astype(ml_dtypes.bfloat16),
            "sinw": sin_win.astype(ml_dtypes.bfloat16),
            "mid": mid,
            "mrd": mrd,
            "mre": mre_z if chunk == 0 else mre_n,
            "biases": bias,
        })
    return in_maps


def _install_ntff_shim():
    """bass_utils wants antenv.axon_hooks for trace=True under axon; this
    environment lacks that module, so synthesize it from the boot helper."""
    import sys
    import types
    if "antenv.axon_hooks" in sys.modules:
        return
    try:
        from trn_agent_boot.trn_boot import _ntff_profile_via_ctypes
        hook = _ntff_profile_via_ctypes("/opt/axon/libaxon_pjrt.so")
    except Exception:
        hook = None
    mod = types.ModuleType("antenv.axon_hooks")
    mod.get_axon_ntff_profile_hook = lambda: hook
    mod.set_axon_ntff_profile_hook = lambda h: None
    sys.modules["antenv.axon_hooks"] = mod


def run(x, w_qkv, w_o, trace=False):
    if "nc" not in _CACHE:
        _CACHE["nc"] = _build()
    nc = _CACHE["nc"]
    in_maps = _prepare_in_maps(np.asarray(x), np.asarray(w_qkv),
                               np.asarray(w_o))
    if trace:
        _install_ntff_shim()
    try:
        res = run_bass_kernel_spmd(nc, in_maps, list(range(NCORES)),
                                   trace=trace)
    except Exception:
        if not trace:
            raise
        res = run_bass_kernel_spmd(nc, in_maps, list(range(NCORES)),
                                   trace=False)
    y = np.empty((B, S, E), dtype=np.float32)
    for c in range(NCORES):
        b, chunk = divmod(c, 4)
        y[b, chunk * CH:(chunk + 1) * CH, :] = res.results[c]["yt"].T
    return y, res


def kernel(x, w_qkv, w_o):
    y, _ = run(x, w_qkv, w_o, trace=False)
    return y
